# revision 1
# baseline (speedup 1.0000x reference)
"""Trainium2 Bass kernel for nn_AtomicNeuralNetwork (species-routed per-atom MLP).

Math (per frame n, atom a with species s = numbers[a]):
    h1 = silu(W1[s].T x + b1[s]);  h2 = silu(W2[s].T h1 + b2[s]);  out = W3[s].T h2 + b3[s]
Shapes: N=4096 frames, A=256 atoms, D_IN=39, H=50, S=8 species.

Strategy:
  - Data parallel over frames: 512 frames per NeuronCore x 8 cores.
  - Host groups atoms into species-pure "packs" of 4 (padding each species
    with duplicate atoms to a multiple of 4; dups discarded on unshard).
  - Per pack, the 3 layers run as PE matmuls with the per-species weights
    stationary and frames on the moving axis, packed 4-at-a-time into the
    128x128 array with tile_position (64x64 quadrants for L1/L2; K=50,M=1
    at col positions {0,32,64,96} for L3).
  - silu + bias fused on ScalarE straight out of PSUM ([128,1024] per pack).
  - b3 + PSUM evacuation on VectorE; strided-partition DMA to DRAM.
  - Everything bf16 on the matmul path (PSUM accumulates fp32); desc is
    downcast to bf16 on the host, which also halves the HBM traffic.
"""

import sys

for _p in ("/opt/trn_rl_repo",):
    if _p not in sys.path:
        sys.path.append(_p)

import numpy as np
import ml_dtypes

import concourse.bass as bass  # noqa: F401  (import keeps bass registered)
import concourse.mybir as mybir
import concourse.tile as tile
from concourse import bacc
from concourse import bass_utils

N, A, D, H, S = 4096, 256, 39, 50, 8
NCORES = 8
NF = N // NCORES            # frames per core
MM_DT = mybir.dt.bfloat16   # matmul operand dtype
NP_MM = ml_dtypes.bfloat16

# stash for test harness introspection (not used by grading)
LAST = {}


def _pack_atoms(species):
    """Group atom indices into species-pure packs of 4, padding each species
    with duplicated atoms. Returns (slot_atoms [NSLOT], pack_species [NPACK])."""
    slot_atoms = []
    pack_species = []
    for s in range(S):
        idxs = np.nonzero(species == s)[0].tolist()
        if not idxs:
            continue
        while len(idxs) % 4:
            idxs.append(idxs[-1])
        for i in range(0, len(idxs), 4):
            slot_atoms.extend(idxs[i:i + 4])
            pack_species.append(s)
    return np.array(slot_atoms), np.array(pack_species)


def _build_program(pack_species, npack):
    nc = bacc.Bacc("TRN2", target_bir_lowering=False, debug=False)

    desc_in = nc.dram_tensor("desc_in", [npack, 2, D, 2 * NF], MM_DT, kind="ExternalInput")
    w1_in = nc.dram_tensor("w1_in", [128, S * H], MM_DT, kind="ExternalInput")
    w2_in = nc.dram_tensor("w2_in", [128, S * H], MM_DT, kind="ExternalInput")
    w3_in = nc.dram_tensor("w3_in", [128, S], MM_DT, kind="ExternalInput")
    b1_in = nc.dram_tensor("b1_in", [128, S], mybir.dt.float32, kind="ExternalInput")
    b2_in = nc.dram_tensor("b2_in", [128, S], mybir.dt.float32, kind="ExternalInput")
    b3_in = nc.dram_tensor("b3_in", [128, S], mybir.dt.float32, kind="ExternalInput")
    out = nc.dram_tensor("out", [4 * npack, NF], mybir.dt.float32, kind="ExternalOutput")

    Silu = mybir.ActivationFunctionType.Silu

    with tile.TileContext(nc) as tc:
        with (
            tc.tile_pool(name="const", bufs=1) as cpool,
            tc.tile_pool(name="dt", bufs=8) as dpool,
            tc.tile_pool(name="h1p", bufs=3) as h1pool,
            tc.tile_pool(name="h2p", bufs=3) as h2pool,
            tc.tile_pool(name="op", bufs=3) as opool,
            tc.tile_pool(name="ps1", bufs=2, space="PSUM") as ps1pool,
            tc.tile_pool(name="ps2", bufs=1, space="PSUM") as ps2pool,
            tc.tile_pool(name="ps3", bufs=2, space="PSUM") as ps3pool,
        ):
            w1 = cpool.tile([128, S * H], MM_DT)
            w2 = cpool.tile([128, S * H], MM_DT)
            w3 = cpool.tile([128, S], MM_DT)
            b1 = cpool.tile([128, S], mybir.dt.float32)
            b2 = cpool.tile([128, S], mybir.dt.float32)
            b3 = cpool.tile([128, S], mybir.dt.float32)
            for t, src in ((w1, w1_in), (w2, w2_in), (w3, w3_in),
                           (b1, b1_in), (b2, b2_in), (b3, b3_in)):
                nc.sync.dma_start(t[:], src[:])

            for p in range(npack):
                s = int(pack_species[p])
                sl = slice(s * H, (s + 1) * H)

                dt_t = dpool.tile([128, 2 * NF], MM_DT)
                # block 0 -> partitions 0..38 (slots 4p,4p+1 side by side),
                # block 1 -> partitions 64..102 (slots 4p+2,4p+3)
                nc.sync.dma_start(dt_t[0:D, :], desc_in[p, 0])
                nc.sync.dma_start(dt_t[64:64 + D, :], desc_in[p, 1])

                ps1 = ps1pool.tile([128, 2 * NF], mybir.dt.float32)
                nc.tensor.matmul(ps1[0:H, 0:NF], w1[0:D, sl], dt_t[0:D, 0:NF],
                                 start=True, stop=True, tile_position=(0, 0))
                nc.tensor.matmul(ps1[64:64 + H, 0:NF], w1[0:D, sl], dt_t[0:D, NF:2 * NF],
                                 start=True, stop=True, tile_position=(0, 64))
                nc.tensor.matmul(ps1[0:H, NF:2 * NF], w1[64:64 + D, sl], dt_t[64:64 + D, 0:NF],
                                 start=True, stop=True, tile_position=(64, 0))
                nc.tensor.matmul(ps1[64:64 + H, NF:2 * NF], w1[64:64 + D, sl], dt_t[64:64 + D, NF:2 * NF],
                                 start=True, stop=True, tile_position=(64, 64))

                h1 = h1pool.tile([128, 2 * NF], MM_DT)
                nc.scalar.activation(h1[:], ps1[:], Silu, bias=b1[:, s:s + 1])

                ps2 = ps2pool.tile([128, 2 * NF], mybir.dt.float32)
                nc.tensor.matmul(ps2[0:H, 0:NF], w2[0:H, sl], h1[0:H, 0:NF],
                                 start=True, stop=True, tile_position=(0, 0))
                nc.tensor.matmul(ps2[64:64 + H, 0:NF], w2[0:H, sl], h1[0:H, NF:2 * NF],
                                 start=True, stop=True, tile_position=(0, 64))
                nc.tensor.matmul(ps2[0:H, NF:2 * NF], w2[64:64 + H, sl], h1[64:64 + H, 0:NF],
                                 start=True, stop=True, tile_position=(64, 0))
                nc.tensor.matmul(ps2[64:64 + H, NF:2 * NF], w2[64:64 + H, sl], h1[64:64 + H, NF:2 * NF],
                                 start=True, stop=True, tile_position=(64, 64))

                h2 = h2pool.tile([128, 2 * NF], MM_DT)
                nc.scalar.activation(h2[:], ps2[:], Silu, bias=b2[:, s:s + 1])

                # L3: psum partitions {0,32,64,96} <- slots {4p,4p+1,4p+2,4p+3}
                ps3 = ps3pool.tile([128, NF], mybir.dt.float32)
                nc.tensor.matmul(ps3[0:1, :], w3[0:H, s:s + 1], h2[0:H, 0:NF],
                                 start=True, stop=True, tile_position=(0, 0))
                nc.tensor.matmul(ps3[32:33, :], w3[0:H, s:s + 1], h2[0:H, NF:2 * NF],
                                 start=True, stop=True, tile_position=(0, 32))
                nc.tensor.matmul(ps3[64:65, :], w3[64:64 + H, s:s + 1], h2[64:64 + H, 0:NF],
                                 start=True, stop=True, tile_position=(64, 64))
                nc.tensor.matmul(ps3[96:97, :], w3[64:64 + H, s:s + 1], h2[64:64 + H, NF:2 * NF],
                                 start=True, stop=True, tile_position=(64, 96))

                o = opool.tile([128, NF], mybir.dt.float32)
                nc.vector.tensor_scalar_add(o[:], ps3[:], b3[:, s:s + 1])

                src = o[:].rearrange("(a p) f -> a p f", p=32)[:, 0, :]
                nc.sync.dma_start(out[4 * p:4 * p + 4, :], src)

    nc.compile()
    return nc


def _host_inputs(desc, numbers, W1, b1, W2, b2, W3, b3):
    desc = np.asarray(desc, dtype=np.float32)
    numbers = np.asarray(numbers).astype(np.int64)
    W1 = np.asarray(W1, np.float32); b1 = np.asarray(b1, np.float32)
    W2 = np.asarray(W2, np.float32); b2 = np.asarray(b2, np.float32)
    W3 = np.asarray(W3, np.float32); b3 = np.asarray(b3, np.float32)

    slot_atoms, pack_species = _pack_atoms(numbers)
    npack = len(pack_species)
    nslot = 4 * npack

    # weight / bias SBUF images
    w1img = np.zeros((128, S * H), np.float32)
    w2img = np.zeros((128, S * H), np.float32)
    w3img = np.zeros((128, S), np.float32)
    b1img = np.zeros((128, S), np.float32)
    b2img = np.zeros((128, S), np.float32)
    b3img = np.zeros((128, S), np.float32)
    for s in range(S):
        sl = slice(s * H, (s + 1) * H)
        w1img[0:D, sl] = W1[s]; w1img[64:64 + D, sl] = W1[s]
        w2img[0:H, sl] = W2[s]; w2img[64:64 + H, sl] = W2[s]
        w3img[0:H, s] = W3[s, :, 0]; w3img[64:64 + H, s] = W3[s, :, 0]
        b1img[0:H, s] = b1[s]; b1img[64:64 + H, s] = b1[s]
        b2img[0:H, s] = b2[s]; b2img[64:64 + H, s] = b2[s]
        b3img[[0, 32, 64, 96], s] = b3[s, 0]

    wmaps = {
        "w1_in": w1img.astype(NP_MM), "w2_in": w2img.astype(NP_MM),
        "w3_in": w3img.astype(NP_MM),
        "b1_in": b1img, "b2_in": b2img, "b3_in": b3img,
    }

    # per-core desc in device layout [npack, 2, D, 2*NF]
    in_maps = []
    for c in range(NCORES):
        dc = desc[c * NF:(c + 1) * NF][:, slot_atoms, :]      # [NF, NSLOT, D]
        dc = np.ascontiguousarray(dc.transpose(1, 2, 0)).astype(NP_MM)  # [NSLOT, D, NF]
        dc = dc.reshape(npack, 2, 2, D, NF).transpose(0, 1, 3, 2, 4)    # [P,2,D,2,NF]
        dc = np.ascontiguousarray(dc).reshape(npack, 2, D, 2 * NF)
        in_maps.append({"desc_in": dc, **wmaps})
    return in_maps, slot_atoms, pack_species, npack, nslot


def kernel(desc, numbers, W1, b1, W2, b2, W3, b3):
    in_maps, slot_atoms, pack_species, npack, nslot = _host_inputs(
        desc, numbers, W1, b1, W2, b2, W3, b3)

    nc = _build_program(pack_species, npack)

    last_err = None
    for _attempt in range(3):
        try:
            res = bass_utils.run_bass_kernel_spmd(
                nc, in_maps, core_ids=list(range(NCORES)))
            break
        except Exception as e:  # transient axon terminal failures
            last_err = e
            import time
            time.sleep(20)
    else:
        raise last_err

    LAST.update(nc=nc, in_maps=in_maps, res=res, npack=npack)

    out = np.empty((N, A), np.float32)
    for c in range(NCORES):
        oc = res.results[c]["out"]          # [4*npack, NF]
        out[c * NF:(c + 1) * NF, slot_atoms] = oc.T
    return out


# revision 3
# speedup vs baseline: 1.3274x; 1.3274x over previous
"""Trainium2 Bass kernel for nn_AtomicNeuralNetwork (species-routed per-atom MLP).

Math (per frame n, atom a with species s = numbers[a]):
    h1 = silu(W1[s].T x + b1[s]);  h2 = silu(W2[s].T h1 + b2[s]);  out = W3[s].T h2 + b3[s]
Shapes: N=4096 frames, A=256 atoms, D_IN=39, H=50, S=8 species.

Strategy:
  - Data parallel over frames: 512 frames per NeuronCore x 8 cores.
  - Host groups atoms into species-pure "packs" of 4 (padding each species
    with duplicate atoms to a multiple of 4; dups discarded on unshard), and
    packs into "groups" of 8 for DMA batching (~640KB per transfer).
  - Per pack, the 3 layers run as PE matmuls with the per-species weights
    stationary and frames on the moving axis, packed 4-at-a-time into the
    128x128 array with tile_position (64x64 quadrants for L1/L2; K=50,M=1
    at col positions {0,32,64,96} for L3).
  - silu + bias fused on ScalarE straight out of PSUM ([128,1024] per pack).
  - b3 + PSUM evacuation on VectorE into a per-group output tile; one
    strided-partition DMA per group to DRAM.
  - Everything bf16 on the matmul path (PSUM accumulates fp32); desc is
    downcast to bf16 on the host, which also halves the HBM traffic.
"""

import sys

for _p in ("/opt/trn_rl_repo",):
    if _p not in sys.path:
        sys.path.append(_p)

import numpy as np
import ml_dtypes

import concourse.bass as bass  # noqa: F401
import concourse.mybir as mybir
import concourse.tile as tile
from concourse import bacc
from concourse import bass_utils

N, A, D, H, S = 4096, 256, 39, 50, 8
NCORES = 8
NF = N // NCORES            # frames per core
GRP = 8                     # packs per DMA group
MM_DT = mybir.dt.bfloat16
NP_MM = ml_dtypes.bfloat16

LAST = {}


def _pack_atoms(species):
    """Group atom indices into species-pure packs of 4, padding each species
    with duplicated atoms. Returns (slot_atoms [4*NPACK], pack_species [NPACK])."""
    slot_atoms = []
    pack_species = []
    for s in range(S):
        idxs = np.nonzero(species == s)[0].tolist()
        if not idxs:
            continue
        while len(idxs) % 4:
            idxs.append(idxs[-1])
        for i in range(0, len(idxs), 4):
            slot_atoms.extend(idxs[i:i + 4])
            pack_species.append(s)
    return np.array(slot_atoms), np.array(pack_species)


def _groups(npack):
    return [(g, min(GRP, npack - g * GRP)) for g in range((npack + GRP - 1) // GRP)]


def _build_program(pack_species, npack, repeat=0):
    import contextlib

    nc = bacc.Bacc("TRN2", target_bir_lowering=False, debug=False)

    groups = _groups(npack)
    ngrp = len(groups)

    # desc_in[g] block rh (0/1) holds, for each partition q<39, the row
    # [pack j | colhalf ch | frame n] as contiguous GRP*2*NF elements.
    desc_in = nc.dram_tensor("desc_in", [ngrp, 2, D, GRP * 2 * NF], MM_DT, kind="ExternalInput")
    w1_in = nc.dram_tensor("w1_in", [128, S * H], MM_DT, kind="ExternalInput")
    w2_in = nc.dram_tensor("w2_in", [128, S * H], MM_DT, kind="ExternalInput")
    w3_in = nc.dram_tensor("w3_in", [128, S], MM_DT, kind="ExternalInput")
    b1_in = nc.dram_tensor("b1_in", [128, S], mybir.dt.float32, kind="ExternalInput")
    b2_in = nc.dram_tensor("b2_in", [128, S], mybir.dt.float32, kind="ExternalInput")
    b3_in = nc.dram_tensor("b3_in", [128, S], mybir.dt.float32, kind="ExternalInput")
    # out[g, a, j, :] = pack (g*GRP+j), atom-slot a (psum partition 32a)
    out = nc.dram_tensor("out", [ngrp, 4, GRP, NF], mybir.dt.float32, kind="ExternalOutput")

    Silu = mybir.ActivationFunctionType.Silu

    with tile.TileContext(nc) as tc:
        with (
            tc.tile_pool(name="const", bufs=1) as cpool,
            tc.tile_pool(name="dt", bufs=3) as dpool,
            tc.tile_pool(name="h1p", bufs=3) as h1pool,
            tc.tile_pool(name="h2p", bufs=3) as h2pool,
            tc.tile_pool(name="op", bufs=2) as opool,
            tc.tile_pool(name="ps1", bufs=2, space="PSUM") as ps1pool,
            tc.tile_pool(name="ps2", bufs=1, space="PSUM") as ps2pool,
            tc.tile_pool(name="ps3", bufs=2, space="PSUM") as ps3pool,
        ):
            w1 = cpool.tile([128, S * H], MM_DT)
            w2 = cpool.tile([128, S * H], MM_DT)
            w3 = cpool.tile([128, S], MM_DT)
            b1 = cpool.tile([128, S], mybir.dt.float32)
            b2 = cpool.tile([128, S], mybir.dt.float32)
            b3 = cpool.tile([128, S], mybir.dt.float32)
            for t, src in ((w1, w1_in), (w2, w2_in), (w3, w3_in),
                           (b1, b1_in), (b2, b2_in), (b3, b3_in)):
                nc.sync.dma_start(t[:], src[:])

            loop_cm = tc.For_i(0, repeat, 1) if repeat else contextlib.nullcontext()
            with loop_cm:
                for g, gn in groups:
                    gw = gn * 2 * NF
                    dt_t = dpool.tile([128, GRP * 2 * NF], MM_DT)
                    nc.sync.dma_start(dt_t[0:D, 0:gw], desc_in[g, 0, :, 0:gw])
                    nc.sync.dma_start(dt_t[64:64 + D, 0:gw], desc_in[g, 1, :, 0:gw])

                    o = opool.tile([128, GRP * NF], mybir.dt.float32)

                    for j in range(gn):
                        p = g * GRP + j
                        s = int(pack_species[p])
                        sl = slice(s * H, (s + 1) * H)
                        c0, c1, c2 = 2 * j * NF, (2 * j + 1) * NF, (2 * j + 2) * NF

                        ps1 = ps1pool.tile([128, 2 * NF], mybir.dt.float32)
                        nc.tensor.matmul(ps1[0:H, 0:NF], w1[0:D, sl], dt_t[0:D, c0:c1],
                                         start=True, stop=True, tile_position=(0, 0))
                        nc.tensor.matmul(ps1[64:64 + H, 0:NF], w1[0:D, sl], dt_t[0:D, c1:c2],
                                         start=True, stop=True, tile_position=(0, 64))
                        nc.tensor.matmul(ps1[0:H, NF:2 * NF], w1[64:64 + D, sl], dt_t[64:64 + D, c0:c1],
                                         start=True, stop=True, tile_position=(64, 0))
                        nc.tensor.matmul(ps1[64:64 + H, NF:2 * NF], w1[64:64 + D, sl], dt_t[64:64 + D, c1:c2],
                                         start=True, stop=True, tile_position=(64, 64))

                        h1 = h1pool.tile([128, 2 * NF], MM_DT)
                        nc.scalar.activation(h1[:], ps1[:], Silu, bias=b1[:, s:s + 1])

                        ps2 = ps2pool.tile([128, 2 * NF], mybir.dt.float32)
                        nc.tensor.matmul(ps2[0:H, 0:NF], w2[0:H, sl], h1[0:H, 0:NF],
                                         start=True, stop=True, tile_position=(0, 0))
                        nc.tensor.matmul(ps2[64:64 + H, 0:NF], w2[0:H, sl], h1[0:H, NF:2 * NF],
                                         start=True, stop=True, tile_position=(0, 64))
                        nc.tensor.matmul(ps2[0:H, NF:2 * NF], w2[64:64 + H, sl], h1[64:64 + H, 0:NF],
                                         start=True, stop=True, tile_position=(64, 0))
                        nc.tensor.matmul(ps2[64:64 + H, NF:2 * NF], w2[64:64 + H, sl], h1[64:64 + H, NF:2 * NF],
                                         start=True, stop=True, tile_position=(64, 64))

                        h2 = h2pool.tile([128, 2 * NF], MM_DT)
                        nc.scalar.activation(h2[:], ps2[:], Silu, bias=b2[:, s:s + 1])

                        ps3 = ps3pool.tile([128, NF], mybir.dt.float32)
                        nc.tensor.matmul(ps3[0:1, :], w3[0:H, s:s + 1], h2[0:H, 0:NF],
                                         start=True, stop=True, tile_position=(0, 0))
                        nc.tensor.matmul(ps3[32:33, :], w3[0:H, s:s + 1], h2[0:H, NF:2 * NF],
                                         start=True, stop=True, tile_position=(0, 32))
                        nc.tensor.matmul(ps3[64:65, :], w3[64:64 + H, s:s + 1], h2[64:64 + H, 0:NF],
                                         start=True, stop=True, tile_position=(64, 64))
                        nc.tensor.matmul(ps3[96:97, :], w3[64:64 + H, s:s + 1], h2[64:64 + H, NF:2 * NF],
                                         start=True, stop=True, tile_position=(64, 96))

                        nc.vector.tensor_scalar_add(o[:, j * NF:(j + 1) * NF], ps3[:],
                                                    b3[:, s:s + 1])

                    src = o[:, 0:gn * NF].rearrange("(a p) (j f) -> a p j f", p=32, f=NF)[:, 0]
                    nc.sync.dma_start(out[g, :, 0:gn, :], src)

    nc.compile()
    return nc


def _host_inputs(desc, numbers, W1, b1, W2, b2, W3, b3):
    desc = np.asarray(desc, dtype=np.float32)
    numbers = np.asarray(numbers).astype(np.int64)
    W1 = np.asarray(W1, np.float32); b1 = np.asarray(b1, np.float32)
    W2 = np.asarray(W2, np.float32); b2 = np.asarray(b2, np.float32)
    W3 = np.asarray(W3, np.float32); b3 = np.asarray(b3, np.float32)

    slot_atoms, pack_species = _pack_atoms(numbers)
    npack = len(pack_species)
    nslot = 4 * npack
    groups = _groups(npack)
    ngrp = len(groups)

    w1img = np.zeros((128, S * H), np.float32)
    w2img = np.zeros((128, S * H), np.float32)
    w3img = np.zeros((128, S), np.float32)
    b1img = np.zeros((128, S), np.float32)
    b2img = np.zeros((128, S), np.float32)
    b3img = np.zeros((128, S), np.float32)
    for s in range(S):
        sl = slice(s * H, (s + 1) * H)
        w1img[0:D, sl] = W1[s]; w1img[64:64 + D, sl] = W1[s]
        w2img[0:H, sl] = W2[s]; w2img[64:64 + H, sl] = W2[s]
        w3img[0:H, s] = W3[s, :, 0]; w3img[64:64 + H, s] = W3[s, :, 0]
        b1img[0:H, s] = b1[s]; b1img[64:64 + H, s] = b1[s]
        b2img[0:H, s] = b2[s]; b2img[64:64 + H, s] = b2[s]
        b3img[[0, 32, 64, 96], s] = b3[s, 0]

    wmaps = {
        "w1_in": w1img.astype(NP_MM), "w2_in": w2img.astype(NP_MM),
        "w3_in": w3img.astype(NP_MM),
        "b1_in": b1img, "b2_in": b2img, "b3_in": b3img,
    }

    # device desc layout: [ngrp, 2(rowhalf), D, GRP*2*NF] where the last axis
    # is (pack_in_group j, colhalf ch, frame n); slot = 4*pack + 2*rh + ch.
    npack_pad = ngrp * GRP
    in_maps = []
    for c in range(NCORES):
        dc = desc[c * NF:(c + 1) * NF][:, slot_atoms, :]                  # [NF, NSLOT, D]
        dc = np.ascontiguousarray(dc.transpose(1, 2, 0)).astype(NP_MM)    # [NSLOT, D, NF]
        if npack_pad != npack:
            pad = np.zeros((4 * (npack_pad - npack), D, NF), NP_MM)
            dc = np.concatenate([dc, pad], axis=0)
        dc = dc.reshape(ngrp, GRP, 2, 2, D, NF)      # g, j, rh, ch, q, n
        dc = dc.transpose(0, 2, 4, 1, 3, 5)          # g, rh, q, j, ch, n
        dc = np.ascontiguousarray(dc).reshape(ngrp, 2, D, GRP * 2 * NF)
        in_maps.append({"desc_in": dc, **wmaps})
    return in_maps, slot_atoms, pack_species, npack, nslot


def kernel(desc, numbers, W1, b1, W2, b2, W3, b3):
    in_maps, slot_atoms, pack_species, npack, nslot = _host_inputs(
        desc, numbers, W1, b1, W2, b2, W3, b3)

    nc = _build_program(pack_species, npack)

    last_err = None
    for _attempt in range(3):
        try:
            res = bass_utils.run_bass_kernel_spmd(
                nc, in_maps, core_ids=list(range(NCORES)))
            break
        except Exception as e:  # transient axon terminal failures
            last_err = e
            import time
            time.sleep(20)
    else:
        raise last_err

    LAST.update(nc=nc, in_maps=in_maps, res=res, npack=npack)

    out = np.empty((N, A), np.float32)
    for c in range(NCORES):
        oc = res.results[c]["out"]                   # [ngrp, 4, GRP, NF]
        oc = oc.transpose(0, 2, 1, 3).reshape(-1, NF)  # slot-major [ngrp*GRP*4, NF]
        out[c * NF:(c + 1) * NF, slot_atoms] = oc[:nslot].T
    return out
